# revision 15
# baseline (speedup 1.0000x reference)
"""EuclideanAttention Trainium2 kernel (v4).

Sharding (8 cores = 2 batches x 4 head-groups of 4 heads, Megatron-style
column/row parallel): each core computes, for its (batch b, head group g):
  qT,kT = (x W_{q,k})^T in [e, s] layout (65-partition augmented tiles:
          rows 0-63 data, row 64 ones / -A*|k|^2); v natural [s, e] bf16
  S^T[j,i] = A*(2 q_i.k_j - |k_j|^2)    (A = 2^7/ln2 folded into the aug
          rows so both exp paths below need no extra scaling work)
  attn^T = exp(S^T/A) in bf16, computed two ways, split across engines:
    - ACT tiles: native Exp activation with scale=1/A
    - DVE tiles: bf16-domain Schraudolph: bitcast_i16(max(S^T + B, 0))
      (one fused tensor_scalar add+max; clamp-to-0 gives exact underflow)
  AV + softmax sums via augmented v (ones column -> row/col 64 = sums)
  y_partial = vals^T.T @ W_o[row block]; host sums the 4 row-parallel
  partials per batch (the Megatron all-reduce), adds b_o.

All big matmuls run in f32r except attention AV (bf16 weights, bf16 v —
measured rel-err contribution ~1e-3, tolerance 2e-2). AV_REORIENT=True
computes vals[i-part, e] with at as stationary (N=65/instr, half the PE
cycles of the N=512 orientation) + PE transpose back via identity; set
False for the v3 orientation (vaug stationary, N=512) if LoadStationary
overhead on real HW eats the reorient win.
"""

import sys

if "/opt/trn_rl_repo" not in sys.path:
    sys.path.insert(0, "/opt/trn_rl_repo")

import numpy as np

import concourse.bacc as bacc
import concourse.mybir as mybir
from concourse.tile import TileContext
from concourse.bass_utils import run_bass_kernel_spmd

F32 = mybir.dt.float32
F32R = mybir.dt.float32r
BF16 = mybir.dt.bfloat16
I16 = mybir.dt.int16
U16 = mybir.dt.uint16
U32 = mybir.dt.uint32

S = 2048
D = 1024
HD = 64
NH = 4
EC = NH * HD  # 256
ST = S // 128
DT = D // 128
NCORES = 8

A_EXP = float(2.0**7 / np.log(2.0))  # 184.665
B_EXP = 16256.0 - 7.4
AV_REORIENT = True
# jt tiles assigned to the DVE fast-exp path (rest go to ACT native exp)
DVE_JT = (1, 3, 5, 7, 9, 11, 13)

_CACHED = {}
TRACE = False
LAST_RESULT = None


def build_program(repeat=1):
    nc = bacc.Bacc("TRN2", target_bir_lowering=False, debug=False)
    xt_d = nc.dram_tensor("xt", [D, S], F32R, kind="ExternalInput")
    wq_d = nc.dram_tensor("wq", [D, EC], F32R, kind="ExternalInput")
    wk_d = nc.dram_tensor("wk", [D, EC], F32R, kind="ExternalInput")
    wv_d = nc.dram_tensor("wv", [D, EC], F32R, kind="ExternalInput")
    wo_d = nc.dram_tensor("wo", [EC, D], F32R, kind="ExternalInput")
    eye_d = nc.dram_tensor("eye", [128, 128], BF16, kind="ExternalInput")
    y_d = nc.dram_tensor("y", [S, D], F32, kind="ExternalOutput")
    with TileContext(nc) as tc:
        for _ in range(repeat):
            _one_pass(nc, tc, xt_d, wq_d, wk_d, wv_d, wo_d, eye_d, y_d)
    nc.compile()
    return nc


def _one_pass(nc, tc, xt_d, wq_d, wk_d, wv_d, wo_d, eye_d, y_d):
    EXP = mybir.ActivationFunctionType.Exp
    CPY = mybir.ActivationFunctionType.Copy
    MUL = mybir.AluOpType.mult
    ADD = mybir.AluOpType.add
    MAX = mybir.AluOpType.max

    with tc.tile_pool(name="persist", bufs=1) as pp:
        qaug = [
            pp.tile([65, S], F32R, tag=f"qaug{h}", name=f"qaug{h}")
            for h in range(NH)
        ]
        kaug = [
            pp.tile([65, S], F32R, tag=f"kaug{h}", name=f"kaug{h}")
            for h in range(NH)
        ]
        vaug = pp.tile([128, ST, NH, HD + 1], BF16, tag="vaug")
        ones64 = pp.tile([64, 1], F32R, tag="ones64")
        eye_sb = pp.tile([128, 128], BF16, tag="eye")
        nc.vector.memset(ones64[:].bitcast(U32), 0x3F800000)
        nc.sync.dma_start(eye_sb[:], eye_d[:, :])
        for h in range(NH):
            nc.gpsimd.memset(qaug[h][64:65, :].bitcast(U32), 0x3F800000)
        nc.gpsimd.memset(vaug[:, :, :, HD].bitcast(U16), 0x3F80)

        # ---- projections, sb-major so compute pipelines with the x DMA:
        # for each 512-col s-block: q/k proj + v proj + -A*|k|^2 row, while
        # the next s-block's xT tiles stream in.
        with (
            tc.tile_pool(name="xtp", bufs=1) as xp,
            tc.tile_pool(name="wqkv", bufs=1) as wqk,
            tc.tile_pool(name="k2p", bufs=2) as k2p,
            tc.tile_pool(name="psPR", bufs=2, space="PSUM") as psPR,
            tc.tile_pool(name="psVP", bufs=2, space="PSUM") as psVP,
            tc.tile_pool(name="psKS", bufs=2, space="PSUM") as psKS,
        ):
            # DMA order matters: q/k weights + the first s-block of x first
            # (they gate the first matmul), wv before v-proj needs it, the
            # rest of x streams behind the sb loop's compute.
            w_r = {
                nm: wqk.tile([128, DT, EC], F32R, tag=f"w_r{nm}", name=f"wr{nm}")
                for nm in ("q", "k", "v")
            }
            for nm, wd in (("q", wq_d), ("k", wk_d)):
                nc.sync.dma_start(
                    w_r[nm][:], wd.rearrange("(dt dl) e -> dl dt e", dl=128)
                )
            xT = xp.tile([128, DT, S], F32R, tag="xT")

            def dma_x_sb(sb):
                for dt_ in range(DT):
                    nc.sync.dma_start(
                        xT[:, dt_, sb * 512 : (sb + 1) * 512],
                        xt_d[
                            dt_ * 128 : (dt_ + 1) * 128,
                            sb * 512 : (sb + 1) * 512,
                        ],
                    )

            dma_x_sb(0)
            nc.sync.dma_start(
                w_r["v"][:], wv_d.rearrange("(dt dl) e -> dl dt e", dl=128)
            )
            for sb in range(1, 4):
                dma_x_sb(sb)

            for sb in range(4):
                ssl = slice(sb * 512, (sb + 1) * 512)
                for nm, dest, scl in (
                    ("q", qaug, 2.0 * A_EXP),
                    ("k", kaug, 1.0),
                ):
                    for et in range(EC // 128):
                        ps = psPR.tile([128, 512], F32, tag="projps")
                        for dt_ in range(DT):
                            nc.tensor.matmul(
                                ps[:],
                                w_r[nm][:, dt_, et * 128 : (et + 1) * 128],
                                xT[:, dt_, ssl],
                                start=(dt_ == 0),
                                stop=(dt_ == DT - 1),
                            )
                        for half in range(2):
                            h = et * 2 + half
                            nc.scalar.activation(
                                dest[h][0:64, ssl],
                                ps[half * 64 : (half + 1) * 64, :],
                                CPY,
                                scale=scl,
                            )
                for st in range(sb * 4, sb * 4 + 4):
                    ps = psVP.tile([128, EC], F32, tag="vps")
                    for dt_ in range(DT):
                        nc.tensor.matmul(
                            ps[:],
                            xT[:, dt_, st * 128 : (st + 1) * 128],
                            w_r["v"][:, dt_, :],
                            start=(dt_ == 0),
                            stop=(dt_ == DT - 1),
                        )
                    nc.vector.tensor_copy(
                        out=vaug[:, st, :, 0:HD],
                        in_=ps[:].rearrange("p (h e) -> p h e", h=NH),
                    )
                for h in range(NH):
                    k2 = k2p.tile([64, 512], F32R, tag="k2", name="k2")
                    nc.vector.tensor_tensor(
                        out=k2[:],
                        in0=kaug[h][0:64, ssl],
                        in1=kaug[h][0:64, ssl],
                        op=MUL,
                    )
                    ps = psKS.tile([1, 512], F32, tag="ksps")
                    nc.tensor.matmul(
                        ps[:], ones64[:], k2[:], start=True, stop=True
                    )
                    nc.scalar.activation(
                        kaug[h][64:65, ssl], ps[:], CPY, scale=-A_EXP
                    )

        # ---- attention ----
        with tc.tile_pool(name="latev", bufs=1) as lp:
            valsT = lp.tile([128, EC // 128, S], F32R, tag="valsT")
            wo_r = lp.tile([128, EC // 128, D], F32R, tag="wo_r")
            nc.sync.dma_start(
                wo_r[:], wo_d.rearrange("(et el) f -> el et f", el=128)
            )
            if AV_REORIENT:
                _attn_reorient(nc, tc, qaug, kaug, vaug, eye_sb, valsT,
                               EXP, MUL, ADD, MAX)
            else:
                _attn_v3(nc, tc, qaug, kaug, vaug, valsT, EXP, MUL, ADD, MAX)

            # ---- o_proj ----
            with (
                tc.tile_pool(name="yps", bufs=4, space="PSUM") as psY,
                tc.tile_pool(name="ysb", bufs=4) as ysb,
            ):
                for st in range(ST):
                    for db in range(2):
                        ps = psY.tile([128, 512], F32, tag="yps")
                        for eb in range(EC // 128):
                            nc.tensor.matmul(
                                ps[:],
                                valsT[:, eb, st * 128 : (st + 1) * 128],
                                wo_r[:, eb, db * 512 : (db + 1) * 512],
                                start=(eb == 0),
                                stop=(eb == EC // 128 - 1),
                            )
                        yt = ysb.tile([128, 512], F32, tag="yt")
                        if (st * 2 + db) % 2 == 0:
                            nc.vector.tensor_copy(out=yt[:], in_=ps[:])
                        else:
                            nc.scalar.activation(yt[:], ps[:], CPY)
                        nc.sync.dma_start(
                            y_d[
                                st * 128 : (st + 1) * 128,
                                db * 512 : (db + 1) * 512,
                            ],
                            yt[:],
                        )


def _exp_tile(nc, at, sc, jt, EXP, ADD, MAX):
    """attn^T = exp(sc/A) in bf16; sc holds A*logits.

    Emitted as two 512-wide halves so AV consumers of the first half can
    start ~600ns earlier (the exp latency otherwise exceeds the per-jt PE
    work and stalls the in-order PE). Halves alternate ACT/DVE per jt to
    balance engine load.
    """
    for hf in range(2):
        a = at[:, hf * 512 : (hf + 1) * 512]
        s = sc[:, hf * 512 : (hf + 1) * 512]
        if hf == 1:
            nc.vector.tensor_scalar(
                out=a.bitcast(I16),
                in0=s,
                scalar1=B_EXP,
                scalar2=0.0,
                op0=ADD,
                op1=MAX,
            )
        else:
            nc.scalar.activation(a, s, EXP, scale=1.0 / A_EXP)


def _attn_reorient(nc, tc, qaug, kaug, vaug, eye_sb, valsT, EXP, MUL, ADD, MAX):
    """vals[i-part, e] accumulation: at stationary (N=65), PE transpose back.

    Software-pipelined: AV(jt) is issued during iteration jt+1 so the PE
    never waits in-order on exp(jt); each block's normalize (DVE) is issued
    right after its last AV, and its transposes (PE) are deferred into the
    next block's jt==1 slot so the PE keeps streaming scores meanwhile.
    """
    with (
        tc.tile_pool(name="normp", bufs=2) as np_,
        tc.tile_pool(name="attnp", bufs=3) as ap_,
        tc.tile_pool(name="scps", bufs=2, space="PSUM") as psS,
        tc.tile_pool(name="avps", bufs=3, space="PSUM") as psAV,
        tc.tile_pool(name="trps", bufs=1, space="PSUM") as psT,
    ):
        blocks = [(h, ih) for h in range(NH) for ih in range(2)]
        pending = []  # vsb tiles of the previous block awaiting transpose

        def issue_transposes(h, ih, vsbs):
            psTt = psT.tile([64, 8, 128], BF16, tag="trps")
            for half in range(2):
                for c4 in range(4):
                    nc.tensor.transpose(
                        psTt[:, half * 4 + c4, :],
                        vsbs[half][:, c4, :],
                        eye_sb[:],
                    )
            nc.vector.tensor_copy(
                out=valsT[
                    (h % 2) * 64 : (h % 2) * 64 + 64,
                    h // 2,
                    ih * 1024 : ih * 1024 + 1024,
                ],
                in_=psTt[:].rearrange("p c i -> p (c i)"),
            )

        for h, ih in blocks:
            i0 = ih * 1024
            av2 = [
                psAV.tile([128, 4, HD + 1], F32, tag="avps", name=f"av{i}")
                for i in range(2)
            ]
            at_prev = None
            for jt in range(ST):
                sc = psS.tile([128, 1024], F32, tag="scps")
                for b2 in range(2):
                    nc.tensor.matmul(
                        sc[:, b2 * 512 : (b2 + 1) * 512],
                        kaug[h][:, jt * 128 : (jt + 1) * 128],
                        qaug[h][:, i0 + b2 * 512 : i0 + (b2 + 1) * 512],
                        start=True,
                        stop=True,
                    )
                at = ap_.tile([128, 1024], BF16, tag="attn")
                _exp_tile(nc, at[:], sc[:], jt, EXP, ADD, MAX)
                if jt == 1 and pending:
                    issue_transposes(*pending.pop())
                if at_prev is not None:
                    for c in range(8):
                        nc.tensor.matmul(
                            av2[c // 4][:, c % 4, :],
                            at_prev[:, c * 128 : (c + 1) * 128],
                            vaug[:, jt - 1, h, :],
                            start=(jt - 1 == 0),
                            stop=False,
                        )
                at_prev = at
            for c in range(8):
                nc.tensor.matmul(
                    av2[c // 4][:, c % 4, :],
                    at_prev[:, c * 128 : (c + 1) * 128],
                    vaug[:, ST - 1, h, :],
                    start=False,
                    stop=True,
                )
            # normalize on DVE into SBUF; transposes deferred to next block
            vsbs = []
            for half in range(2):
                av = av2[half]
                rec = np_.tile([128, 4], F32, tag="rec")
                nc.vector.reciprocal(rec[:], av[:, :, HD])
                vsb = np_.tile([128, 4, HD], BF16, tag="vsb", name=f"vsb{half}")
                for c4 in range(4):
                    nc.gpsimd.tensor_scalar_mul(
                        vsb[:, c4, :], av[:, c4, 0:HD], rec[:, c4 : c4 + 1]
                    )
                vsbs.append(vsb)
            pending.append((h, ih, vsbs))
        issue_transposes(*pending.pop())


def _attn_v3(nc, tc, qaug, kaug, vaug, valsT, EXP, MUL, ADD, MAX):
    """v3 orientation: vaug stationary, at moving (N=512)."""
    with (
        tc.tile_pool(name="normp", bufs=1) as np_,
        tc.tile_pool(name="attnp", bufs=3) as ap_,
        tc.tile_pool(name="scps", bufs=2, space="PSUM") as psS,
        tc.tile_pool(name="avps", bufs=2, space="PSUM") as psAV,
    ):
        for h in range(NH):
            for ih in range(2):
                i0 = ih * 1024
                isl = slice(i0, i0 + 1024)
                av = psAV.tile([HD + 1, 1024], F32, tag="avps")
                at_prev = None
                for jt in range(ST):
                    sc = psS.tile([128, 1024], F32, tag="scps")
                    for b2 in range(2):
                        nc.tensor.matmul(
                            sc[:, b2 * 512 : (b2 + 1) * 512],
                            kaug[h][:, jt * 128 : (jt + 1) * 128],
                            qaug[h][:, i0 + b2 * 512 : i0 + (b2 + 1) * 512],
                            start=True,
                            stop=True,
                        )
                    at = ap_.tile([128, 1024], BF16, tag="attn")
                    _exp_tile(nc, at[:], sc[:], jt, EXP, ADD, MAX)
                    if at_prev is not None:
                        for b2 in range(2):
                            nc.tensor.matmul(
                                av[:, b2 * 512 : (b2 + 1) * 512],
                                vaug[:, jt - 1, h, :],
                                at_prev[:, b2 * 512 : (b2 + 1) * 512],
                                start=(jt - 1 == 0),
                                stop=False,
                            )
                    at_prev = at
                for b2 in range(2):
                    nc.tensor.matmul(
                        av[:, b2 * 512 : (b2 + 1) * 512],
                        vaug[:, ST - 1, h, :],
                        at_prev[:, b2 * 512 : (b2 + 1) * 512],
                        start=False,
                        stop=True,
                    )
                avs = np_.tile([HD + 1, 1024], F32, tag="avs")
                nc.vector.tensor_copy(out=avs[:], in_=av[:])
                rec = np_.tile([1, 1024], F32, tag="rec")
                nc.vector.reciprocal(rec[:], avs[HD : HD + 1, :])
                rb = np_.tile([64, 1024], F32, tag="rb")
                nc.gpsimd.partition_broadcast(rb[:], rec[:])
                nc.vector.tensor_tensor(
                    out=valsT[(h % 2) * 64 : (h % 2) * 64 + 64, h // 2, isl],
                    in0=avs[0:HD, :],
                    in1=rb[:],
                    op=MUL,
                )


def _numpy_fallback(x, W_qkv, b_qkv, W_o, b_o):
    B, S_, D_ = x.shape
    H, Hd = 16, 64
    qkv = x.reshape(-1, D_) @ W_qkv + b_qkv
    qkv = qkv.reshape(B, S_, H, 3 * Hd).transpose(0, 2, 1, 3)
    q, k, v = np.split(qkv, 3, axis=-1)
    out = np.empty((B, S_, D_), np.float32)
    for b in range(B):
        for h in range(H):
            qb, kb, vb = q[b, h], k[b, h], v[b, h]
            lg = 2 * qb @ kb.T - (qb * qb).sum(-1)[:, None] - (kb * kb).sum(-1)[None, :]
            lg -= lg.max(-1, keepdims=True)
            w = np.exp(lg)
            w /= w.sum(-1, keepdims=True)
            out[b, :, h * Hd : (h + 1) * Hd] = w @ vb
    return (out.reshape(-1, D_) @ W_o + b_o).reshape(B, S_, D_)


def make_in_maps(x, W_qkv, W_o):
    import ml_dtypes

    Wr = W_qkv.reshape(D, 16, 3, HD)
    xts = [np.ascontiguousarray(x[b].T) for b in range(2)]
    eye = np.eye(128, dtype=ml_dtypes.bfloat16)
    in_maps = []
    for c in range(NCORES):
        b, g = c // 4, c % 4
        e0 = g * EC
        hsl = slice(NH * g, NH * (g + 1))
        in_maps.append(
            {
                "xt": xts[b],
                "wq": np.ascontiguousarray(Wr[:, hsl, 0, :].reshape(D, EC)),
                "wk": np.ascontiguousarray(Wr[:, hsl, 1, :].reshape(D, EC)),
                "wv": np.ascontiguousarray(Wr[:, hsl, 2, :].reshape(D, EC)),
                "wo": np.ascontiguousarray(W_o[e0 : e0 + EC, :]),
                "eye": eye,
            }
        )
    return in_maps


def kernel(x, W_qkv, b_qkv, W_o, b_o):
    x = np.ascontiguousarray(np.asarray(x, dtype=np.float32))
    W_qkv = np.ascontiguousarray(np.asarray(W_qkv, dtype=np.float32))
    b_qkv = np.asarray(b_qkv, dtype=np.float32)
    W_o = np.ascontiguousarray(np.asarray(W_o, dtype=np.float32))
    b_o = np.asarray(b_o, dtype=np.float32)

    if np.any(b_qkv):
        return _numpy_fallback(x, W_qkv, b_qkv, W_o, b_o)

    if "nc" not in _CACHED:
        _CACHED["nc"] = build_program()
    nc = _CACHED["nc"]

    in_maps = make_in_maps(x, W_qkv, W_o)
    kw = {}
    if TRACE:
        kw = dict(trace=True, trace_cores=list(range(NCORES)))
    res = run_bass_kernel_spmd(nc, in_maps, core_ids=list(range(NCORES)), **kw)
    global LAST_RESULT
    LAST_RESULT = res

    out = np.zeros((2, S, D), np.float32)
    for c in range(NCORES):
        out[c // 4] += res.results[c]["y"]
    out += b_o
    return out
